# revision 3
# baseline (speedup 1.0000x reference)
"""Trainium2 Bass kernel for nn_Actor (3-layer MLP actor + reparameterized
sampling + int quantization), data-parallel across 8 NeuronCores.

  h1 = relu(state @ W1.T + b1)        state: [B, 128],  W1: [256, 128]
  h2 = relu(h1 @ W2.T + b2)           W2: [256, 256]
  n  = h2 @ W3.T + b3                 W3: [16, 256]
  x  = n[:, :8] + |n[:, 8:]| * eps
  out = int32(sigmoid(x) * 8 + 1)     (device int cast semantics)

Per-core layout strategy (feature-major activations):
  - state tiles are cast-DMA'd f32->bf16, transposed on PE via identity matmul
  - L1/L2 run feature-major (weights stationary, batch as the moving free dim)
  - L3 runs batch-major: lhsT = h2T column slices, rhs = W3.T -> n [batch, 16]
  - final stage is elementwise on [128, 64] tiles, sigmoid on ACT,
    f32->int32 cast on DVE (hardware round-to-nearest-even, matching the
    XLA/neuron reference semantics for .astype(int32))
"""

import numpy as np
import ml_dtypes

B, D, H, A = 262144, 128, 256, 8
NCORES = 8
ROWS = B // NCORES        # 32768 rows per core
CHUNK = 512               # batch rows per compute chunk
GROUP = 4                 # chunks per DMA group (2048 rows)

_BUILD_CACHE = {}


def _legalize_waits(nc, max_waits=1):
    """Walrus in this container rejects >1 sync wait per instruction; peel
    extra waits onto preceding same-engine nops (semantically identical:
    the engine blocks at the same program point either way)."""
    from concourse import mybir

    uid = 0
    for fn in nc.m.functions:
        for blk in fn.blocks:
            insts = blk.instructions
            out = []
            for inst in insts:
                si = inst.sync_info
                if si is not None and si.on_wait and len(si.on_wait) > max_waits:
                    waits = list(si.on_wait)
                    extra, keep = waits[:-max_waits], waits[-max_waits:]
                    for w in extra:
                        nop = mybir.InstNoOp(name=f"I-wsplit-{uid}", ins=[], outs=[])
                        uid += 1
                        nop.engine = inst.engine
                        nop.sync_info = mybir.SyncInfo(on_wait=[w], on_update=[])
                        out.append(nop)
                    inst.sync_info = mybir.SyncInfo(
                        on_wait=keep, on_update=list(si.on_update or [])
                    )
                out.append(inst)
            insts[:] = out


def build(rows=ROWS):
    import concourse.bass as bass
    import concourse.tile as tile
    from concourse import mybir
    from contextlib import ExitStack


    f32 = mybir.dt.float32
    bf16 = mybir.dt.bfloat16
    i32 = mybir.dt.int32
    AF = mybir.ActivationFunctionType
    OP = mybir.AluOpType

    nc = bass.Bass()
    state_e = nc.declare_dram_parameter("state", [rows, D], f32, isOutput=False)
    eps_e = nc.declare_dram_parameter("eps", [rows, A], f32, isOutput=False)
    w1t_e = nc.declare_dram_parameter("w1t", [D, H], bf16, isOutput=False)
    w2t_e = nc.declare_dram_parameter("w2t", [H, H], bf16, isOutput=False)
    w3t_e = nc.declare_dram_parameter("w3t", [H, 2 * A], bf16, isOutput=False)
    b1_e = nc.declare_dram_parameter("b1", [H], f32, isOutput=False)
    b2_e = nc.declare_dram_parameter("b2", [H], f32, isOutput=False)
    b3_e = nc.declare_dram_parameter("b3r", [128, 4 * 2 * A], f32, isOutput=False)
    id_e = nc.declare_dram_parameter("ident", [128, 128], bf16, isOutput=False)
    out_e = nc.declare_dram_parameter("out", [rows, A], i32, isOutput=True)

    n_groups = rows // (CHUNK * GROUP)
    GROWS = CHUNK * GROUP  # rows per group

    with tile.TileContext(nc) as tc, ExitStack() as ctx:
        consts = ctx.enter_context(tc.tile_pool(name="consts", bufs=1))
        inp = ctx.enter_context(tc.tile_pool(name="inp", bufs=3))
        epsp = ctx.enter_context(tc.tile_pool(name="epsp", bufs=2))
        stp = ctx.enter_context(tc.tile_pool(name="stp", bufs=3))
        actp = ctx.enter_context(tc.tile_pool(name="actp", bufs=2))
        finp = ctx.enter_context(tc.tile_pool(name="finp", bufs=2))
        outp = ctx.enter_context(tc.tile_pool(name="outp", bufs=2))
        ps_t = ctx.enter_context(tc.tile_pool(name="ps_t", bufs=2, space="PSUM"))
        ps_h1 = ctx.enter_context(tc.tile_pool(name="ps_h1", bufs=1, space="PSUM"))
        ps_h2 = ctx.enter_context(tc.tile_pool(name="ps_h2", bufs=1, space="PSUM"))
        ps_n = ctx.enter_context(tc.tile_pool(name="ps_n", bufs=2, space="PSUM"))

        # ---- constants ----
        ident = consts.tile([128, 128], bf16)
        nc.sync.dma_start(ident[:], id_e[:])
        w1t = consts.tile([128, H], bf16)              # [128, 256]
        nc.sync.dma_start(w1t[:], w1t_e[:])
        w2t = consts.tile([128, 2 * H], bf16)          # [(k p), m] -> [p, (k m)]
        nc.sync.dma_start(w2t[:], w2t_e.rearrange("(k p) m -> p k m", p=128))
        w3t = consts.tile([128, 2 * 2 * A], bf16)      # [p, (k f)]
        nc.sync.dma_start(w3t[:], w3t_e.rearrange("(k p) f -> p k f", p=128))
        b1 = consts.tile([128, 2], f32)
        nc.sync.dma_start(b1[:], b1_e.rearrange("(h p) -> p h", p=128))
        b2 = consts.tile([128, 2], f32)
        nc.sync.dma_start(b2[:], b2_e.rearrange("(h p) -> p h", p=128))
        b3 = consts.tile([128, 4 * 2 * A], f32)        # b3 tiled 4x along free
        nc.sync.dma_start(b3[:], b3_e[:])

        st_v = state_e.rearrange("(g c t p) d -> g p c t d", p=128, t=GROUP, c=GROUP)
        eps_v = eps_e.rearrange("(g c t p) f -> g p c t f", p=128, t=GROUP, c=GROUP)
        out_v = out_e.rearrange("(g c t p) f -> g p c t f", p=128, t=GROUP, c=GROUP)

        for g in range(n_groups):
            # ---- group DMA loads ----
            st_in = inp.tile([128, GROWS], bf16)       # cast f32->bf16 in DMA
            nc.gpsimd.dma_start(
                st_in[:].rearrange("p (c t d) -> p c t d", c=GROUP, t=GROUP),
                st_v[g],
            )
            eps_sb = epsp.tile([128, GROUP * GROUP * A], f32)
            nc.sync.dma_start(
                eps_sb[:].rearrange("p (c t f) -> p c t f", c=GROUP, t=GROUP),
                eps_v[g],
            )
            y_sb = outp.tile([128, GROUP * GROUP * A], i32)

            for c in range(GROUP):
                # ---- transpose state chunk on PE: stT = state_tile.T ----
                stT_ps = ps_t.tile([128, CHUNK], f32)
                for t in range(GROUP):
                    nc.tensor.matmul(
                        stT_ps[:, 128 * t : 128 * (t + 1)],
                        lhsT=st_in[:, c * CHUNK + 128 * t : c * CHUNK + 128 * (t + 1)],
                        rhs=ident[:],
                    )
                stT = stp.tile([128, CHUNK], bf16)
                # split PSUM->SBUF copy across ACT and DVE
                nc.scalar.activation(stT[:, :256], stT_ps[:, :256], AF.Copy)
                nc.vector.tensor_copy(stT[:, 256:], stT_ps[:, 256:])

                # ---- L1: h1T[half] = W1T[:, half].T @ stT ----
                h1_ps = ps_h1.tile([128, 2 * CHUNK], f32)
                for m in range(2):
                    nc.tensor.matmul(
                        h1_ps[:, CHUNK * m : CHUNK * (m + 1)],
                        lhsT=w1t[:, 128 * m : 128 * (m + 1)],
                        rhs=stT[:],
                    )
                h1 = actp.tile([128, 2 * CHUNK], bf16, tag="h1")
                for m in range(2):
                    nc.scalar.activation(
                        h1[:, CHUNK * m : CHUNK * (m + 1)],
                        h1_ps[:, CHUNK * m : CHUNK * (m + 1)],
                        AF.Relu,
                        bias=b1[:, m : m + 1],
                    )

                # ---- L2: h2T[m] = sum_k W2T[k][:, m].T @ h1T[k] ----
                h2_ps = ps_h2.tile([128, 2 * CHUNK], f32)
                for m in range(2):
                    for k in range(2):
                        nc.tensor.matmul(
                            h2_ps[:, CHUNK * m : CHUNK * (m + 1)],
                            lhsT=w2t[:, H * k + 128 * m : H * k + 128 * (m + 1)],
                            rhs=h1[:, CHUNK * k : CHUNK * (k + 1)],
                            start=(k == 0),
                            stop=(k == 1),
                        )
                h2 = actp.tile([128, 2 * CHUNK], bf16, tag="h2")
                for m in range(2):
                    # max(in + b2, 0) with bf16 cast, on DVE
                    nc.vector.tensor_scalar(
                        h2[:, CHUNK * m : CHUNK * (m + 1)],
                        h2_ps[:, CHUNK * m : CHUNK * (m + 1)],
                        b2[:, m : m + 1],
                        0.0,
                        OP.add,
                        OP.max,
                    )

                # ---- L3 (batch-major): n[t] = sum_k h2T[k][:,t].T @ W3T[k] ----
                n_ps = ps_n.tile([128, GROUP * 2 * A], f32)
                for t in range(GROUP):
                    for k in range(2):
                        nc.tensor.matmul(
                            n_ps[:, 2 * A * t : 2 * A * (t + 1)],
                            lhsT=h2[:, CHUNK * k + 128 * t : CHUNK * k + 128 * (t + 1)],
                            rhs=w3t[:, 2 * A * k : 2 * A * (k + 1)],
                            start=(k == 0),
                            stop=(k == 1),
                        )

                # ---- final stage on [128, GROUP*16] ----
                nb = finp.tile([128, GROUP * 2 * A], f32, tag="nb")
                nc.vector.tensor_tensor(nb[:], n_ps[:], b3[:], OP.add)
                nb4 = nb[:].rearrange("p (t f) -> p t f", t=GROUP)
                sabs = finp.tile([128, GROUP * A], f32, tag="sabs")
                nc.scalar.activation(
                    sabs[:].rearrange("p (t f) -> p t f", t=GROUP),
                    nb4[:, :, A : 2 * A],
                    AF.Abs,
                )
                x = finp.tile([128, GROUP * A], f32, tag="x")
                nc.vector.tensor_tensor(
                    x[:], sabs[:], eps_sb[:, GROUP * A * c : GROUP * A * (c + 1)],
                    OP.mult,
                )
                nc.vector.tensor_tensor(
                    x[:].rearrange("p (t f) -> p t f", t=GROUP),
                    x[:].rearrange("p (t f) -> p t f", t=GROUP),
                    nb4[:, :, 0:A],
                    OP.add,
                )
                sig = finp.tile([128, GROUP * A], f32, tag="sig")
                nc.scalar.activation(sig[:], x[:], AF.Sigmoid)
                nc.vector.tensor_scalar(
                    y_sb[:, GROUP * A * c : GROUP * A * (c + 1)],
                    sig[:],
                    8.0,
                    1.0,
                    OP.mult,
                    OP.add,
                )

            nc.sync.dma_start(
                out_v[g],
                y_sb[:].rearrange("p (c t f) -> p c t f", c=GROUP, t=GROUP),
            )

    _legalize_waits(nc)
    return nc


def _get_nc(rows=ROWS):
    if rows not in _BUILD_CACHE:
        _BUILD_CACHE[rows] = build(rows)
    return _BUILD_CACHE[rows]


def _prep_weights(W1, b1, W2, b2, W3, b3):
    bf = ml_dtypes.bfloat16
    w1t = np.ascontiguousarray(W1.T).astype(bf)                 # [128, 256]
    w2t = np.ascontiguousarray(W2.T).astype(bf)                 # [256, 256]
    w3t = np.ascontiguousarray(W3.T).astype(bf)                 # [256, 16]
    b3r = np.tile(np.asarray(b3, np.float32)[None, :], (128, 4))  # [128, 64]
    ident = np.eye(128, dtype=bf)
    return {
        "w1t": w1t,
        "w2t": w2t,
        "w3t": w3t,
        "b1": np.ascontiguousarray(b1, np.float32),
        "b2": np.ascontiguousarray(b2, np.float32),
        "b3r": np.ascontiguousarray(b3r),
        "ident": ident,
    }


def run(inputs, rows=ROWS, trace=False, **kw):
    """inputs: full-size dict from setup_inputs(). Returns (out, results)."""
    from concourse.bass_utils import run_bass_kernel_spmd

    nc = _get_nc(rows)
    shared = _prep_weights(
        inputs["W1"], inputs["b1"], inputs["W2"], inputs["b2"],
        inputs["W3"], inputs["b3"],
    )
    state = np.ascontiguousarray(np.asarray(inputs["state"], np.float32))
    eps = np.ascontiguousarray(np.asarray(inputs["eps"], np.float32))
    in_maps = []
    for c in range(NCORES):
        in_maps.append({
            "state": state[c * ROWS : c * ROWS + rows],
            "eps": eps[c * ROWS : c * ROWS + rows],
            **shared,
        })
    res = run_bass_kernel_spmd(nc, in_maps, list(range(NCORES)), trace=trace, **kw)
    out = np.concatenate([res.results[c]["out"] for c in range(NCORES)], axis=0)
    return out, res


def kernel(state, W1, b1, W2, b2, W3, b3, eps):
    out_shards, _ = run({
        "state": state, "W1": W1, "b1": b1, "W2": W2, "b2": b2,
        "W3": W3, "b3": b3, "eps": eps,
    })
    return out_shards
